# revision 46
# baseline (speedup 1.0000x reference)
"""Segment mean-pool (LocalPooling1D) Trainium2 Bass kernel.

x [32, 8192, 256] f32, x_pos [32, 65] sorted int32 boundaries -> y [32, 64, 256].
y[b, j] = mean(x[b, x_pos[b,j]:x_pos[b,j+1]]), empty segments -> 0.

Strategy: data-parallel over batch, 4 rows per core on 8 cores; the kernel is
HBM-bandwidth-bound, so everything is built around a clean ~432 GB/s x stream.

Token t of a row maps to SBUF partition p = t // 64, free-slot q = t % 64, so
every partition's x data is one contiguous 64 KB HBM chunk -> uniform 16 KB
DMA descriptors that the HWDGE deals evenly over all 16 SDMA engines at line
rate. The 0/1 segment-indicator ind[p, q, j] = (pos[j] <= 64p + q < pos[j+1])
is built on the DVE per x-chunk (so the first matmul starts a few us in), from
a tiny [128, 64] q-iota and a [128, P] broadcast of pos done on the (idle at
startup) TensorEngine as ones[1,128].T @ pos[1,P] - avoiding the gpsimd
PartitionBroadcast custom op, whose first use stalls ~10us on a Q7 library
reload. Segment sums accumulate on the PE as psum += ind_q.T @ x_q in
float32r (1 cycle/row at N=256, 4x faster than fp32; ind is exactly 0/1 so
only x's low mantissa bits are lost - rel err ~1e-4, tolerance 2e-2).

No SWDGE (gpsimd) DMAs anywhere: SWDGE descriptor-ring traffic contends with
SDMA engines 7/15 and engine 15 was observed as a ~20% straggler on cores
where it is active. The last x chunk of the last row is split into small
pieces so the post-stream matmul burst is short, and all four rows' outputs
are stored with one HWDGE DMA at the very end.
"""

import os
import sys

import numpy as np

sys.path.insert(0, "/opt/trn_rl_repo")

import concourse.bacc as bacc
import concourse.bass as bass
import concourse.tile as tile
from concourse import mybir
from concourse.bass_utils import run_bass_kernel_spmd

dt = mybir.dt
Alu = mybir.AluOpType

# Problem constants (hardcoded per harness contract).
B, T, C, P = 32, 8192, 256, 65
NSEG = P - 1
NCORES = 8
R = B // NCORES          # batch rows per core
NPART = 128              # SBUF partitions
QTOK = T // NPART        # 64 tokens per partition (contiguous in HBM)

CFG = {
    "chunkq": int(os.environ.get("KB_CHUNKQ", "16")),      # q-slices per x DMA
    "x_bufs": int(os.environ.get("KB_XBUFS", "8")),
    "ind_bufs": int(os.environ.get("KB_INDBUFS", "6")),
    "s_bufs": int(os.environ.get("KB_SBUFS", "3")),
    "psum_bufs": int(os.environ.get("KB_PSUMBUFS", "2")),
    "dual_dma": os.environ.get("KB_DUALDMA", "1") == "1",
}


def build_program(cfg=CFG):
    chunkq = cfg["chunkq"]
    nchunk = QTOK // chunkq

    nc = bacc.Bacc("TRN2", target_bir_lowering=False, debug=False)

    # float32r: same bit layout as f32; enables the 1-cycle/row PE matmul mode
    # (vs 4 for fp32). The BIR verifier requires matmul operand producers to
    # declare f32r output, so x is f32r end-to-end (DMA is then a plain copy).
    x_d = nc.dram_tensor("x", [R, T, C], dt.float32r, kind="ExternalInput")
    pos_d = nc.dram_tensor("x_pos", [R, P], dt.int32, kind="ExternalInput")
    y_d = nc.dram_tensor("y", [R, NSEG, C], dt.float32, kind="ExternalOutput")

    with tile.TileContext(nc) as tc:
        with (
            tc.tile_pool(name="const", bufs=1) as constp,
            tc.tile_pool(name="xp", bufs=cfg["x_bufs"]) as xp,
            tc.tile_pool(name="sp", bufs=cfg["s_bufs"]) as sp,
            tc.tile_pool(name="indp", bufs=cfg["ind_bufs"]) as indp,
            tc.tile_pool(name="smallp", bufs=R) as smallp,
            tc.tile_pool(name="outp", bufs=2) as outp,
            tc.tile_pool(name="psp", bufs=cfg["psum_bufs"], space="PSUM") as psp,
            tc.tile_pool(name="pspos", bufs=1, space="PSUM") as pspos,
            tc.tile_pool(name="xtailp", bufs=2) as xtailp,
            tc.tile_pool(name="stailp", bufs=2) as stailp,
            tc.tile_pool(name="indtailp", bufs=2) as indtailp,
        ):
            # q (token index within partition) along the free axis: [128, 64].
            q_sm = constp.tile([NPART, QTOK], dt.float32)
            nc.gpsimd.iota(q_sm[:], pattern=[[1, QTOK]], base=0,
                           channel_multiplier=0, allow_small_or_imprecise_dtypes=True)
            # 64*p as a per-partition scalar (<= 8128, exact in f32).
            p64_iota = constp.tile([NPART, 1], dt.float32)
            nc.gpsimd.iota(p64_iota[:], pattern=[[1, 1]], base=0, channel_multiplier=QTOK,
                           allow_small_or_imprecise_dtypes=True)
            ones_row = constp.tile([1, NPART], dt.float32)
            nc.gpsimd.iota(ones_row[:], pattern=[[0, NPART]], base=1,
                           channel_multiplier=0, allow_small_or_imprecise_dtypes=True)

            ones1 = constp.tile([1, 1], dt.float32, tag="ones1")
            nc.gpsimd.iota(ones1[:], pattern=[[0, 1]], base=1,
                           channel_multiplier=0, allow_small_or_imprecise_dtypes=True)

            # All pos rows in ONE single-descriptor 1 KB DMA on the scalar
            # queue. The sync queue starts directly with x chunk 0: one
            # queue's 128 descriptors already saturate all 16 SDMA engines
            # during the ramp, and HWDGE descriptor generation appears to be
            # a shared resource - anything generated ahead of chunk 0 just
            # delays the stream.
            pos_all = smallp.tile([1, R * P], dt.int32, tag="posall")
            nc.scalar.dma_start(
                pos_all[:].rearrange("one (r p) -> one r p", r=R), pos_d[:, :])
            pos_rows = [pos_all[:, r * P : (r + 1) * P] for r in range(R)]

            # Per row: pos broadcast to 128 partitions on the PE
            # (ones[1,128].T @ pos[1,P]), and segment counts computed in the
            # free axis then transposed to [NSEG, 1] with a K=1 matmul
            # (cnt[1,64].T @ ones[1,1]) - no transposed pos DMAs (those cost
            # 256 sub-512B descriptors each on the scalar queue).
            pos_bs, recips = [], []
            for r in range(R):
                posf_row = smallp.tile([1, P], dt.float32, tag="posf")
                nc.vector.tensor_copy(posf_row[:], pos_rows[r])
                ps_pos = pspos.tile([NPART, P], dt.float32)
                nc.tensor.matmul(ps_pos[:], ones_row[:], posf_row[:],
                                 start=True, stop=True)
                pos_b = smallp.tile([NPART, P], dt.float32, tag="posb")
                nc.vector.tensor_copy(pos_b[:], ps_pos[:])
                pos_bs.append(pos_b)

                cnt_row = smallp.tile([1, NSEG], dt.float32, tag="cntrow")
                nc.vector.tensor_tensor(
                    cnt_row[:], posf_row[:, 1:P], posf_row[:, 0:NSEG], op=Alu.subtract)
                ps_cnt = pspos.tile([NSEG, 1], dt.float32, tag="cntT")
                nc.tensor.matmul(ps_cnt[:], cnt_row[:], ones1[:],
                                 start=True, stop=True)
                cntc = smallp.tile([NSEG, 1], dt.float32, tag="cntc")
                nc.vector.tensor_scalar(cntc[:], ps_cnt[:], 1.0, None, op0=Alu.max)
                recip = smallp.tile([NSEG, 1], dt.float32, tag="recip")
                nc.vector.reciprocal(recip[:], cntc[:])
                recips.append(recip)

            # All four rows' outputs accumulate here; one HWDGE store at the
            # end (no per-row SWDGE stores).
            y_all = outp.tile([NSEG, R * C], dt.float32)

            for r in range(R):
                pos_b = pos_bs[r]
                ps = psp.tile([NSEG, C], dt.float32)
                # Row as [128 partitions, 64*256]: partition p's line is the
                # contiguous HBM range of tokens [64p, 64p+64).
                xr = x_d[r].rearrange("(p q) c -> p (q c)", p=NPART)
                # The very last chunk gates the kernel tail (DMA-completion
                # latency + matmul burst + scale + store all serialize after
                # it): split it into small pieces so the post-stream burst is
                # short.
                if r == R - 1:
                    tailq = max(4, chunkq // 2)
                    qsteps = [chunkq] * (nchunk - 1) + [tailq] * (chunkq // tailq)
                else:
                    qsteps = [chunkq] * nchunk
                q0 = 0
                for ci, cq in enumerate(qsteps):
                    tail = cq != chunkq
                    # S[p, k, j] = (pos[j] <= 64p + q), q = q0 + k.
                    S_c = (stailp if tail else sp).tile([NPART, cq, P], dt.float32, tag="sall")
                    nc.vector.scalar_tensor_tensor(
                        S_c[:],
                        pos_b[:, None, :].broadcast_to((NPART, cq, P)),
                        p64_iota[:],
                        q_sm[:, q0 : q0 + cq, None].broadcast_to((NPART, cq, P)),
                        op0=Alu.subtract,
                        op1=Alu.is_le,
                    )
                    # ind[p, k, j] = S[p, k, j] - S[p, k, j+1]
                    ind_c = (indtailp if tail else indp).tile([NPART, cq, NSEG], dt.float32r, tag="ind")
                    nc.vector.tensor_tensor(
                        ind_c[:], S_c[:, :, 0:NSEG], S_c[:, :, 1:P], op=Alu.subtract
                    )

                    xt = (xtailp if tail else xp).tile([NPART, cq * C], dt.float32r, tag="x")
                    if r == 0 and ci == 0:
                        # First chunk split across both queues as 64-partition
                        # halves: each HWDGE ring generates only 64
                        # descriptors before bytes start moving, halving the
                        # cold-start latency to the first descriptor.
                        nc.sync.dma_start(xt[0:64, :], xr[0:64, q0 * C : (q0 + cq) * C])
                        nc.scalar.dma_start(xt[64:NPART, :], xr[64:NPART, q0 * C : (q0 + cq) * C])
                    else:
                        eng = nc.scalar if (cfg["dual_dma"] and ci % 2) else nc.sync
                        eng.dma_start(xt[:], xr[:, q0 * C : (q0 + cq) * C])
                    for k in range(cq):
                        q = q0 + k
                        rhs = xt[:, k * C : (k + 1) * C]
                        lhsT = ind_c[:, k, :]
                        nc.tensor.matmul(
                            ps[:], lhsT, rhs,
                            start=(q == 0), stop=(q == QTOK - 1),
                        )
                    q0 += cq

                out_t = y_all[:, r * C : (r + 1) * C]
                nc.vector.tensor_scalar(out_t, ps[:], recips[r][:], None, op0=Alu.mult)

            # Per-row HWDGE stores, all issued after the last x issue so they
            # block nothing; rows 0-2 complete while the last row still
            # computes, leaving only row 3's 64 KB (+receipt) in the tail.
            for r in range(R):
                eng = nc.scalar if r % 2 else nc.sync
                eng.dma_start(y_d[r], y_all[:, r * C : (r + 1) * C])

    nc.compile()
    return nc


_PROGRAM = None


def _get_program():
    global _PROGRAM
    if _PROGRAM is None:
        _PROGRAM = build_program()
    return _PROGRAM


def kernel(x, x_pos):
    x = np.ascontiguousarray(x, dtype=np.float32)
    x_pos = np.ascontiguousarray(x_pos, dtype=np.int32)
    nc = _get_program()
    in_maps = [
        {"x": x[c * R : (c + 1) * R], "x_pos": x_pos[c * R : (c + 1) * R]}
        for c in range(NCORES)
    ]
    res = run_bass_kernel_spmd(nc, in_maps, list(range(NCORES)))
    y = np.concatenate([res.results[c]["y"] for c in range(NCORES)], axis=0)
    return y.astype(np.float32)


# revision 47
# speedup vs baseline: 1.2113x; 1.2113x over previous
"""Segment mean-pool (LocalPooling1D) Trainium2 Bass kernel.

x [32, 8192, 256] f32, x_pos [32, 65] sorted int32 boundaries -> y [32, 64, 256].
y[b, j] = mean(x[b, x_pos[b,j]:x_pos[b,j+1]]), empty segments -> 0.

Strategy: data-parallel over batch, 4 rows per core on 8 cores; the kernel is
HBM-bandwidth-bound, so everything is built around a clean ~432 GB/s x stream.

Token t of a row maps to SBUF partition p = t // 64, free-slot q = t % 64, so
every partition's x data is one contiguous 64 KB HBM chunk -> uniform 16 KB
DMA descriptors that the HWDGE deals evenly over all 16 SDMA engines at line
rate. The 0/1 segment-indicator ind[p, q, j] = (pos[j] <= 64p + q < pos[j+1])
is built on the DVE per x-chunk (so the first matmul starts a few us in), from
a tiny [128, 64] q-iota and a [128, P] broadcast of pos done on the (idle at
startup) TensorEngine as ones[1,128].T @ pos[1,P] - avoiding the gpsimd
PartitionBroadcast custom op, whose first use stalls ~10us on a Q7 library
reload. Segment sums accumulate on the PE as psum += ind_q.T @ x_q in
float32r (1 cycle/row at N=256, 4x faster than fp32; ind is exactly 0/1 so
only x's low mantissa bits are lost - rel err ~1e-4, tolerance 2e-2).

No SWDGE (gpsimd) DMAs anywhere: SWDGE descriptor-ring traffic contends with
SDMA engines 7/15 and engine 15 was observed as a ~20% straggler on cores
where it is active. The last x chunk of the last row is split into small
pieces so the post-stream matmul burst is short, and all four rows' outputs
are stored with one HWDGE DMA at the very end.
"""

import os
import sys

import numpy as np

sys.path.insert(0, "/opt/trn_rl_repo")

import concourse.bacc as bacc
import concourse.bass as bass
import concourse.tile as tile
from concourse import mybir
from concourse.bass_utils import run_bass_kernel_spmd

dt = mybir.dt
Alu = mybir.AluOpType

# Problem constants (hardcoded per harness contract).
B, T, C, P = 32, 8192, 256, 65
NSEG = P - 1
NCORES = 8
R = B // NCORES          # batch rows per core
NPART = 128              # SBUF partitions
QTOK = T // NPART        # 64 tokens per partition (contiguous in HBM)

CFG = {
    "chunkq": int(os.environ.get("KB_CHUNKQ", "16")),      # q-slices per x DMA
    "x_bufs": int(os.environ.get("KB_XBUFS", "8")),
    "ind_bufs": int(os.environ.get("KB_INDBUFS", "6")),
    "s_bufs": int(os.environ.get("KB_SBUFS", "3")),
    "psum_bufs": int(os.environ.get("KB_PSUMBUFS", "2")),
    "dual_dma": os.environ.get("KB_DUALDMA", "1") == "1",
}


def build_program(cfg=CFG):
    chunkq = cfg["chunkq"]
    nchunk = QTOK // chunkq

    nc = bacc.Bacc("TRN2", target_bir_lowering=False, debug=False)

    # float32r: same bit layout as f32; enables the 1-cycle/row PE matmul mode
    # (vs 4 for fp32). The BIR verifier requires matmul operand producers to
    # declare f32r output, so x is f32r end-to-end (DMA is then a plain copy).
    x_d = nc.dram_tensor("x", [R, T, C], dt.float32r, kind="ExternalInput")
    pos_d = nc.dram_tensor("x_pos", [R, P], dt.int32, kind="ExternalInput")
    y_d = nc.dram_tensor("y", [R, NSEG, C], dt.float32, kind="ExternalOutput")

    with tile.TileContext(nc) as tc:
        with (
            tc.tile_pool(name="const", bufs=1) as constp,
            tc.tile_pool(name="xp", bufs=cfg["x_bufs"]) as xp,
            tc.tile_pool(name="sp", bufs=cfg["s_bufs"]) as sp,
            tc.tile_pool(name="indp", bufs=cfg["ind_bufs"]) as indp,
            tc.tile_pool(name="smallp", bufs=R) as smallp,
            tc.tile_pool(name="outp", bufs=2) as outp,
            tc.tile_pool(name="psp", bufs=cfg["psum_bufs"], space="PSUM") as psp,
            tc.tile_pool(name="pspos", bufs=1, space="PSUM") as pspos,
            tc.tile_pool(name="xtailp", bufs=2) as xtailp,
            tc.tile_pool(name="stailp", bufs=2) as stailp,
            tc.tile_pool(name="indtailp", bufs=2) as indtailp,
        ):
            # q (token index within partition) along the free axis: [128, 64].
            q_sm = constp.tile([NPART, QTOK], dt.float32)
            nc.gpsimd.iota(q_sm[:], pattern=[[1, QTOK]], base=0,
                           channel_multiplier=0, allow_small_or_imprecise_dtypes=True)
            # 64*p as a per-partition scalar (<= 8128, exact in f32).
            p64_iota = constp.tile([NPART, 1], dt.float32)
            nc.gpsimd.iota(p64_iota[:], pattern=[[1, 1]], base=0, channel_multiplier=QTOK,
                           allow_small_or_imprecise_dtypes=True)
            ones_row = constp.tile([1, NPART], dt.float32)
            nc.gpsimd.iota(ones_row[:], pattern=[[0, NPART]], base=1,
                           channel_multiplier=0, allow_small_or_imprecise_dtypes=True)

            ones1 = constp.tile([1, 1], dt.float32, tag="ones1")
            nc.gpsimd.iota(ones1[:], pattern=[[0, 1]], base=1,
                           channel_multiplier=0, allow_small_or_imprecise_dtypes=True)

            # All pos rows in ONE single-descriptor 1 KB DMA on the scalar
            # queue. The sync queue starts directly with x chunk 0: one
            # queue's 128 descriptors already saturate all 16 SDMA engines
            # during the ramp, and HWDGE descriptor generation appears to be
            # a shared resource - anything generated ahead of chunk 0 just
            # delays the stream.
            pos_all = smallp.tile([1, R * P], dt.int32, tag="posall")
            nc.scalar.dma_start(
                pos_all[:].rearrange("one (r p) -> one r p", r=R), pos_d[:, :])
            pos_rows = [pos_all[:, r * P : (r + 1) * P] for r in range(R)]

            # Per row: pos broadcast to 128 partitions on the PE
            # (ones[1,128].T @ pos[1,P]), and segment counts computed in the
            # free axis then transposed to [NSEG, 1] with a K=1 matmul
            # (cnt[1,64].T @ ones[1,1]) - no transposed pos DMAs (those cost
            # 256 sub-512B descriptors each on the scalar queue).
            pos_bs, recips = [], []
            for r in range(R):
                posf_row = smallp.tile([1, P], dt.float32, tag="posf")
                nc.vector.tensor_copy(posf_row[:], pos_rows[r])
                ps_pos = pspos.tile([NPART, P], dt.float32)
                nc.tensor.matmul(ps_pos[:], ones_row[:], posf_row[:],
                                 start=True, stop=True)
                pos_b = smallp.tile([NPART, P], dt.float32, tag="posb")
                nc.vector.tensor_copy(pos_b[:], ps_pos[:])
                pos_bs.append(pos_b)

                cnt_row = smallp.tile([1, NSEG], dt.float32, tag="cntrow")
                nc.vector.tensor_tensor(
                    cnt_row[:], posf_row[:, 1:P], posf_row[:, 0:NSEG], op=Alu.subtract)
                ps_cnt = pspos.tile([NSEG, 1], dt.float32, tag="cntT")
                nc.tensor.matmul(ps_cnt[:], cnt_row[:], ones1[:],
                                 start=True, stop=True)
                cntc = smallp.tile([NSEG, 1], dt.float32, tag="cntc")
                nc.vector.tensor_scalar(cntc[:], ps_cnt[:], 1.0, None, op0=Alu.max)
                recip = smallp.tile([NSEG, 1], dt.float32, tag="recip")
                nc.vector.reciprocal(recip[:], cntc[:])
                recips.append(recip)

            # All four rows' outputs accumulate here; one HWDGE store at the
            # end (no per-row SWDGE stores).
            y_all = outp.tile([NSEG, R * C], dt.float32)

            for r in range(R):
                pos_b = pos_bs[r]
                ps = psp.tile([NSEG, C], dt.float32)
                # Row as [128 partitions, 64*256]: partition p's line is the
                # contiguous HBM range of tokens [64p, 64p+64).
                xr = x_d[r].rearrange("(p q) c -> p (q c)", p=NPART)
                # The very last chunk gates the kernel tail (DMA-completion
                # latency + matmul burst + scale + store all serialize after
                # it): split it into small pieces so the post-stream burst is
                # short.
                if r == R - 1:
                    tailq = max(4, chunkq // 2)
                    qsteps = [chunkq] * (nchunk - 1) + [tailq] * (chunkq // tailq)
                else:
                    qsteps = [chunkq] * nchunk
                q0 = 0
                for ci, cq in enumerate(qsteps):
                    tail = cq != chunkq
                    # S[p, k, j] = (pos[j] <= 64p + q), q = q0 + k.
                    S_c = (stailp if tail else sp).tile([NPART, cq, P], dt.float32, tag="sall")
                    nc.vector.scalar_tensor_tensor(
                        S_c[:],
                        pos_b[:, None, :].broadcast_to((NPART, cq, P)),
                        p64_iota[:],
                        q_sm[:, q0 : q0 + cq, None].broadcast_to((NPART, cq, P)),
                        op0=Alu.subtract,
                        op1=Alu.is_le,
                    )
                    # ind[p, k, j] = S[p, k, j] - S[p, k, j+1]
                    ind_c = (indtailp if tail else indp).tile([NPART, cq, NSEG], dt.float32r, tag="ind")
                    nc.vector.tensor_tensor(
                        ind_c[:], S_c[:, :, 0:NSEG], S_c[:, :, 1:P], op=Alu.subtract
                    )

                    xt = (xtailp if tail else xp).tile([NPART, cq * C], dt.float32r, tag="x")
                    eng = nc.scalar if (cfg["dual_dma"] and ci % 2) else nc.sync
                    eng.dma_start(xt[:], xr[:, q0 * C : (q0 + cq) * C])
                    for k in range(cq):
                        q = q0 + k
                        rhs = xt[:, k * C : (k + 1) * C]
                        lhsT = ind_c[:, k, :]
                        nc.tensor.matmul(
                            ps[:], lhsT, rhs,
                            start=(q == 0), stop=(q == QTOK - 1),
                        )
                    q0 += cq

                out_t = y_all[:, r * C : (r + 1) * C]
                nc.vector.tensor_scalar(out_t, ps[:], recips[r][:], None, op0=Alu.mult)

            # Per-row HWDGE stores, all issued after the last x issue so they
            # block nothing; rows 0-2 complete while the last row still
            # computes, leaving only row 3's 64 KB (+receipt) in the tail.
            for r in range(R):
                eng = nc.scalar if r % 2 else nc.sync
                eng.dma_start(y_d[r], y_all[:, r * C : (r + 1) * C])

    nc.compile()
    return nc


_PROGRAM = None


def _get_program():
    global _PROGRAM
    if _PROGRAM is None:
        _PROGRAM = build_program()
    return _PROGRAM


def kernel(x, x_pos):
    x = np.ascontiguousarray(x, dtype=np.float32)
    x_pos = np.ascontiguousarray(x_pos, dtype=np.int32)
    nc = _get_program()
    in_maps = [
        {"x": x[c * R : (c + 1) * R], "x_pos": x_pos[c * R : (c + 1) * R]}
        for c in range(NCORES)
    ]
    res = run_bass_kernel_spmd(nc, in_maps, list(range(NCORES)))
    y = np.concatenate([res.results[c]["y"] for c in range(NCORES)], axis=0)
    return y.astype(np.float32)
